# revision 16
# baseline (speedup 1.0000x reference)
"""Trainium2 Bass kernel for nn_Compressor (sparse_attention, hierarchical window MLP).

Reference computation (per batch b, head h):
  windows w=0..510 over k[b,h] (S=8192, D=128), window length 32, stride 16
  x[w, l, :] = k[16w+l, :] + pe[l, :]
  5 stages of pairwise-merge MLP: x <- silu(x.reshape(-1, 256) @ w_down[i].T)
  out[w+1] = x @ w_stop.T   ; out[0] = 0 (prepended zero window)

Sharding: head-parallel across 8 cores (B*H = 32 -> 4 heads/core), weights
replicated, no cross-device comms.

Algebraic optimization: stage-0 operates on adjacent row pairs (s=2t, 2t+1)
and every pair is shared by exactly two windows (stride 16, pair width 2),
always in the same even/odd role.  So
  Z[:, t] = W0_even @ kT[:, 2t] + W0_odd @ kT[:, 2t+1]
is computed once per pair (half the naive stage-0 flops) and the window/
position-dependent part enters only through the bias:
  S0[:, (w, j)] = silu(Z[:, 8w+j] + (W0 @ pe_pair_j))
with the pe-bias folded into the ScalarE activation instruction.

Layout: everything is kept "plane-major" so every matmul moving operand,
every activation input/output, and every copy is contiguous (strided engine
reads cost 2-4x on TRN2):
  ktp[d, l, w]   = bf16 k[16w + l, d]     (l-planes; built by PE transposes
                                           through a host-provided column
                                           permutation matrix P2)
  Z plane e      = psum of W0e @ ktp[:, 2e, :] + W0o @ ktp[:, 2e+1, :]
  s0[d, j, w]    = silu(Zplane(j%8)[:, w + (j>=8)] + pe0[:, j])
  s{i}[d, p, w]  = silu-merged planes, stage i
The final w_stop matmul uses the data as the stationary operand, producing
output already row-major for a clean DMA out.
"""

import numpy as np

B, H, S, D = 2, 16, 8192, 128
BH = B * H
NCORES = 8
HPC = BH // NCORES  # heads per core = 4
NB = (S - 32) // 16 + 1  # 511 sliding windows
NW = NB + 1  # 512 output rows per head (incl. zero window)

# w_stop output chunking: window ranges per PE (stationary) chunk
QRANGES = [(0, 128), (128, 128), (256, 128), (384, 127)]

_BASS_CACHE = {}


def _build_bass():
    import concourse.bacc as bacc
    import concourse.mybir as mybir
    import concourse.tile as tile
    from bass_rust import add_dep_helper

    f32 = mybir.dt.float32
    bf16 = mybir.dt.bfloat16
    SILU = mybir.ActivationFunctionType.Silu

    nc = bacc.Bacc()
    # k4p: host-prepermuted bf16 k; row 512*l + w holds k[16w + l, :], so a
    # single xbar DMA-transpose per head lands the l-planar layout directly.
    k4p = nc.dram_tensor("k4p", [HPC, S, D], bf16, kind="ExternalInput")
    wdt = nc.dram_tensor("wdt", [5, 2, 128, 128], bf16, kind="ExternalInput")
    pe0 = nc.dram_tensor("pe0", [128, 16], f32, kind="ExternalInput")
    wst = nc.dram_tensor("wst", [128, 128], bf16, kind="ExternalInput")
    oqs = [
        [
            nc.dram_tensor(f"o{hh}_{q}", [wq, 128], f32, kind="ExternalOutput")
            for q, (_, wq) in enumerate(QRANGES)
        ]
        for hh in range(HPC)
    ]

    with tile.TileContext(nc) as tc:
        with (
            tc.tile_pool(name="consts", bufs=1) as consts,
            tc.tile_pool(name="ktp", bufs=2) as ktp,
            tc.tile_pool(name="s0p", bufs=2) as s0p,
            tc.tile_pool(name="stp", bufs=2) as stp,
            tc.tile_pool(name="outp", bufs=2) as outp,
            tc.tile_pool(name="tps", bufs=1, space="PSUM") as tps,
            tc.tile_pool(name="zps", bufs=2, space="PSUM") as zps,
            tc.tile_pool(name="sps", bufs=2, space="PSUM") as sps,
            tc.tile_pool(name="ops", bufs=1, space="PSUM") as ops,
        ):
            wd_sb = consts.tile([128, 5, 2, 128], bf16, name="wd_sb")
            nc.sync.dma_start(out=wd_sb, in_=wdt.rearrange("i h k o -> k i h o"))
            pe0_sb = consts.tile([128, 16], f32, name="pe0_sb")
            nc.sync.dma_start(out=pe0_sb, in_=pe0[:])
            wst_sb = consts.tile([128, 128], bf16, name="wst_sb")
            nc.sync.dma_start(out=wst_sb, in_=wst[:])

            # The walrus pipeline fuses a matmul's sem waits into its
            # LDWEIGHTS slot (1 wait); extra waits cost an EventSemaphore
            # instruction.  Absorber matmuls make PE observe each semaphore
            # cheaply first.  They write disjoint 2-col regions of one
            # never-read PSUM bank (no WAW sems between them).
            dummy = tps.tile([128, 512], f32, name="dummy", tag="dummy", bufs=1)
            dummy_ctr = [0]

            def absorb(lhsT, rhs):
                m = dummy_ctr[0]
                dummy_ctr[0] += 1
                dst = dummy[: lhsT.shape[-1], 2 * m : 2 * m + 2]
                return nc.tensor.matmul(dst, lhsT=lhsT, rhs=rhs, start=True, stop=True)

            absorb(wd_sb[:, 0, 0, 0:2], wd_sb[:, 0, 0, 0:2])
            absorb(wst_sb[:, 0:2], wst_sb[:, 0:2])

            def filler():
                # tiny dependency-free matmul; keeps the PE HAM activity
                # window busy so the clock stays at 2.4 GHz through gaps
                nc.tensor.matmul(
                    dummy[0:2, 256:512], lhsT=wd_sb[:, 0, 0, 0:2],
                    rhs=wd_sb[:, 0, :, :], start=True, stop=True,
                )

            def z_phase(hh):
                # one xbar DMA-transpose lands the l-planar kt directly
                ktf = ktp.tile([128, S], bf16, name="ktf")
                nc.sync.dma_start(out=ktf, in_=k4p[hh], transpose=True)
                kt3 = ktf.rearrange("p (l w) -> p l w", w=512)
                s0 = s0p.tile([128, 16, NB], bf16, name="s0")
                abk = absorb(kt3[:, 0, 0:2], kt3[:, 0, 0:2])
                for e in range(8):
                    filler()
                    zp = zps.tile([128, 512], f32, name="zp", tag="zp")
                    mm = nc.tensor.matmul(
                        zp, lhsT=wd_sb[:, 0, 0, :], rhs=kt3[:, 2 * e, :],
                        start=True, stop=False,
                    )
                    if e == 0:
                        add_dep_helper(mm.ins, abk.ins, False,
                                       "absorber before first Z matmul")
                    nc.tensor.matmul(
                        zp, lhsT=wd_sb[:, 0, 1, :], rhs=kt3[:, 2 * e + 1, :],
                        start=False, stop=True,
                    )
                    nc.scalar.activation(
                        out=s0[:, e, :], in_=zp[:, 0:NB], func=SILU,
                        bias=pe0_sb[:, e : e + 1], scale=1.0,
                    )
                    nc.scalar.activation(
                        out=s0[:, e + 8, :], in_=zp[:, 1 : NB + 1], func=SILU,
                        bias=pe0_sb[:, e + 8 : e + 9], scale=1.0,
                    )
                return s0

            def stage_phase(hh, s0):
                prev = s0
                for st in range(1, 5):
                    nj = 16 >> st
                    cur = stp.tile([128, nj, NB], bf16, name=f"s{st}", tag=f"s{st}")
                    filler()
                    for p in range((nj + 1) // 2):
                        npl = min(2, nj - 2 * p)  # planes in this psum tile
                        ps = sps.tile([128, 2, 512], f32, name="ps", tag="sp")
                        for ii in range(npl):
                            i = 2 * p + ii
                            nc.tensor.matmul(
                                ps[:, ii, :NB], lhsT=wd_sb[:, st, 0, :],
                                rhs=prev[:, 2 * i, :],
                                start=True, stop=False,
                            )
                            nc.tensor.matmul(
                                ps[:, ii, :NB], lhsT=wd_sb[:, st, 1, :],
                                rhs=prev[:, 2 * i + 1, :],
                                start=False, stop=True,
                            )
                        # one batched silu over the psum planes
                        nc.scalar.activation(
                            out=cur[:, 2 * p : 2 * p + npl, :],
                            in_=ps[:, :npl, :NB], func=SILU,
                        )
                    prev = cur

                # ---- w_stop with data-stationary -> row-major [w, o] out ----
                s4f = prev[:, 0, :]  # [128, 511]
                outsb = outp.tile([128, 4, 128], f32, name="outsb")
                ab2 = absorb(s4f[:, 0:2], s4f[:, 0:2])
                filler()
                for q, (w0, wq) in enumerate(QRANGES):
                    ps2 = ops.tile([128, 512], f32, name="ps2", tag="op")
                    mmq = nc.tensor.matmul(
                        ps2[:wq, :128],
                        lhsT=s4f[:, w0 : w0 + wq],
                        rhs=wst_sb,
                        start=True, stop=True,
                    )
                    if q == 0:
                        add_dep_helper(mmq.ins, ab2.ins, False,
                                       "absorber before first stop matmul")
                    nc.vector.tensor_copy(out=outsb[:wq, q, :], in_=ps2[:wq, :128])
                    nc.sync.dma_start(out=oqs[hh][q][:], in_=outsb[:wq, q, :])

            # software pipeline: emit Z-phase of head h+1 before the stage
            # phase of head h so PE/ACT always have a head of lookahead
            s0s = {0: z_phase(0)}
            for hh in range(HPC):
                if hh + 1 < HPC:
                    s0s[hh + 1] = z_phase(hh + 1)
                stage_phase(hh, s0s.pop(hh))

    if not nc.is_finalized():
        nc.finalize()
    return nc


def _prep_host_inputs(k, pe, w_down, w_stop):
    import ml_dtypes

    bf16 = ml_dtypes.bfloat16
    k = np.asarray(k, dtype=np.float32)
    pe = np.asarray(pe, dtype=np.float32)
    w_down = np.asarray(w_down, dtype=np.float32)
    w_stop = np.asarray(w_stop, dtype=np.float32)

    # pre-permute rows so row 512*l + w = k[16w + l, :], cast to bf16 (RNE);
    # a single device-side xbar DMA-transpose then yields the l-planar layout
    k4p = np.ascontiguousarray(
        k.reshape(BH, 512, 16, D).transpose(0, 2, 1, 3).reshape(BH, S, D)
    ).astype(bf16)
    # wdt[i, half, d_in, o] = w_down[i][o, 128*half + d_in]
    wdt = np.ascontiguousarray(
        w_down.transpose(0, 2, 1).reshape(5, 2, 128, 128)
    ).astype(bf16)
    # pe0[o, j] = sum_i w_down[0][o, i] * concat(pe[2j], pe[2j+1])[i]
    pe_pairs = pe.reshape(16, 256).astype(np.float64)
    pe0 = (w_down[0].astype(np.float64) @ pe_pairs.T).astype(np.float32)
    wst = np.ascontiguousarray(w_stop.T).astype(bf16)
    return k4p, wdt, pe0, wst


def run(k, pe, w_down, w_stop, trace=False, trace_kwargs=None):
    from concourse.bass_utils import run_bass_kernel_spmd

    k4p, wdt, pe0, wst = _prep_host_inputs(k, pe, w_down, w_stop)

    if "nc" not in _BASS_CACHE:
        _BASS_CACHE["nc"] = _build_bass()
    nc = _BASS_CACHE["nc"]

    in_maps = [
        {
            "k4p": np.ascontiguousarray(k4p[HPC * c : HPC * (c + 1)]),
            "wdt": wdt,
            "pe0": pe0,
            "wst": wst,
        }
        for c in range(NCORES)
    ]
    res = run_bass_kernel_spmd(
        nc, in_maps, core_ids=list(range(NCORES)), trace=trace,
        **(trace_kwargs or {}),
    )
    out = np.empty((BH, NW, D), dtype=np.float32)
    for c in range(NCORES):
        r = res.results[c]
        for hh in range(HPC):
            row = HPC * c + hh
            out[row, 0, :] = 0.0
            at = 1
            for q, (_, wq) in enumerate(QRANGES):
                out[row, at : at + wq, :] = r[f"o{hh}_{q}"]
                at += wq
    out = out.reshape(B, H, NW, D)
    return out, res


def kernel(k, pe, w_down, w_stop):
    out, _ = run(k, pe, w_down, w_stop, trace=False)
    return out


# revision 17
# speedup vs baseline: 1.0865x; 1.0865x over previous
"""Trainium2 Bass kernel for nn_Compressor (sparse_attention, hierarchical window MLP).

Reference computation (per batch b, head h):
  windows w=0..510 over k[b,h] (S=8192, D=128), window length 32, stride 16
  x[w, l, :] = k[16w+l, :] + pe[l, :]
  5 stages of pairwise-merge MLP: x <- silu(x.reshape(-1, 256) @ w_down[i].T)
  out[w+1] = x @ w_stop.T   ; out[0] = 0 (prepended zero window)

Sharding: head-parallel across 8 cores (B*H = 32 -> 4 heads/core), weights
replicated, no cross-device comms.

Algebraic optimization: stage-0 operates on adjacent row pairs (s=2t, 2t+1)
and every pair is shared by exactly two windows (stride 16, pair width 2),
always in the same even/odd role.  So
  Z[:, t] = W0_even @ kT[:, 2t] + W0_odd @ kT[:, 2t+1]
is computed once per pair (half the naive stage-0 flops) and the window/
position-dependent part enters only through the bias:
  S0[:, (w, j)] = silu(Z[:, 8w+j] + (W0 @ pe_pair_j))
with the pe-bias folded into the ScalarE activation instruction.

Layout: everything is kept "plane-major" so every matmul moving operand,
every activation input/output, and every copy is contiguous (strided engine
reads cost 2-4x on TRN2):
  ktp[d, l, w]   = bf16 k[16w + l, d]     (l-planes; built by PE transposes
                                           through a host-provided column
                                           permutation matrix P2)
  Z plane e      = psum of W0e @ ktp[:, 2e, :] + W0o @ ktp[:, 2e+1, :]
  s0[d, j, w]    = silu(Zplane(j%8)[:, w + (j>=8)] + pe0[:, j])
  s{i}[d, p, w]  = silu-merged planes, stage i
The final w_stop matmul uses the data as the stationary operand, producing
output already row-major for a clean DMA out.
"""

import numpy as np

B, H, S, D = 2, 16, 8192, 128
BH = B * H
NCORES = 8
HPC = BH // NCORES  # heads per core = 4
NB = (S - 32) // 16 + 1  # 511 sliding windows
NW = NB + 1  # 512 output rows per head (incl. zero window)

# w_stop output chunking: window ranges per PE (stationary) chunk
QRANGES = [(0, 128), (128, 128), (256, 128), (384, 127)]

_BASS_CACHE = {}


def _build_bass():
    import concourse.bacc as bacc
    import concourse.mybir as mybir
    import concourse.tile as tile
    from bass_rust import add_dep_helper

    f32 = mybir.dt.float32
    bf16 = mybir.dt.bfloat16
    SILU = mybir.ActivationFunctionType.Silu

    nc = bacc.Bacc()
    # k4p: host-prepermuted bf16 k; row 512*l + w holds k[16w + l, :], so a
    # single xbar DMA-transpose per head lands the l-planar layout directly.
    k4p = nc.dram_tensor("k4p", [HPC, S, D], bf16, kind="ExternalInput")
    wdt = nc.dram_tensor("wdt", [5, 2, 128, 128], bf16, kind="ExternalInput")
    pe0 = nc.dram_tensor("pe0", [128, 16], f32, kind="ExternalInput")
    wst = nc.dram_tensor("wst", [128, 128], bf16, kind="ExternalInput")
    oqs = [
        [
            nc.dram_tensor(f"o{hh}_{q}", [wq, 128], f32, kind="ExternalOutput")
            for q, (_, wq) in enumerate(QRANGES)
        ]
        for hh in range(HPC)
    ]

    with tile.TileContext(nc) as tc:
        with (
            tc.tile_pool(name="consts", bufs=1) as consts,
            tc.tile_pool(name="ktp", bufs=2) as ktp,
            tc.tile_pool(name="s0p", bufs=2) as s0p,
            tc.tile_pool(name="stp", bufs=2) as stp,
            tc.tile_pool(name="outp", bufs=2) as outp,
            tc.tile_pool(name="tps", bufs=1, space="PSUM") as tps,
            tc.tile_pool(name="zps", bufs=2, space="PSUM") as zps,
            tc.tile_pool(name="sps", bufs=2, space="PSUM") as sps,
            tc.tile_pool(name="ops", bufs=1, space="PSUM") as ops,
        ):
            wd_sb = consts.tile([128, 5, 2, 128], bf16, name="wd_sb")
            nc.sync.dma_start(out=wd_sb, in_=wdt.rearrange("i h k o -> k i h o"))
            pe0_sb = consts.tile([128, 16], f32, name="pe0_sb")
            nc.sync.dma_start(out=pe0_sb, in_=pe0[:])
            wst_sb = consts.tile([128, 128], bf16, name="wst_sb")
            nc.sync.dma_start(out=wst_sb, in_=wst[:])

            # The walrus pipeline fuses a matmul's sem waits into its
            # LDWEIGHTS slot (1 wait); extra waits cost an EventSemaphore
            # instruction.  Absorber matmuls make PE observe each semaphore
            # cheaply first.  They write disjoint 2-col regions of one
            # never-read PSUM bank (no WAW sems between them).
            dummy = tps.tile([128, 512], f32, name="dummy", tag="dummy", bufs=1)
            dummy_ctr = [0]

            def absorb(lhsT, rhs):
                m = dummy_ctr[0]
                dummy_ctr[0] += 1
                dst = dummy[: lhsT.shape[-1], 2 * m : 2 * m + 2]
                return nc.tensor.matmul(dst, lhsT=lhsT, rhs=rhs, start=True, stop=True)

            absorb(wd_sb[:, 0, 0, 0:2], wd_sb[:, 0, 0, 0:2])
            absorb(wst_sb[:, 0:2], wst_sb[:, 0:2])

            def filler():
                # tiny dependency-free matmul; keeps the PE HAM activity
                # window busy so the clock stays at 2.4 GHz through gaps
                nc.tensor.matmul(
                    dummy[0:2, 256:512], lhsT=wd_sb[:, 0, 0, 0:2],
                    rhs=wd_sb[:, 0, :, :], start=True, stop=True,
                )

            def z_units(hh, s0_out):
                """Generator: per-e units of the shared stage-0 (Z) phase."""
                ktf = ktp.tile([128, S], bf16, name="ktf")
                kt3 = ktf.rearrange("p (l w) -> p l w", w=512)
                # 4 chunked xbar DMA-transposes (4 l-planes each) so the
                # first Z matmuls start ~2.5us after the head's DMA begins
                for c in range(4):
                    nc.sync.dma_start(
                        out=ktf[:, 2048 * c : 2048 * (c + 1)],
                        in_=k4p[hh, 2048 * c : 2048 * (c + 1), :],
                        transpose=True,
                    )
                s0 = s0p.tile([128, 16, NB], bf16, name="s0")
                s0_out[hh] = s0
                for e in range(8):
                    if e % 2 == 0:
                        # observe this chunk's DMA lane on PE
                        absorb(kt3[:, 2 * e, 0:2], kt3[:, 2 * e, 0:2])
                    filler()
                    zp = zps.tile([128, 512], f32, name="zp", tag="zp")
                    nc.tensor.matmul(
                        zp, lhsT=wd_sb[:, 0, 0, :], rhs=kt3[:, 2 * e, :],
                        start=True, stop=False,
                    )
                    nc.tensor.matmul(
                        zp, lhsT=wd_sb[:, 0, 1, :], rhs=kt3[:, 2 * e + 1, :],
                        start=False, stop=True,
                    )
                    nc.scalar.activation(
                        out=s0[:, e, :], in_=zp[:, 0:NB], func=SILU,
                        bias=pe0_sb[:, e : e + 1], scale=1.0,
                    )
                    nc.scalar.activation(
                        out=s0[:, e + 8, :], in_=zp[:, 1 : NB + 1], func=SILU,
                        bias=pe0_sb[:, e + 8 : e + 9], scale=1.0,
                    )
                    yield

            def stage_units(hh, s0):
                """Generator: per-silu-group units of stages 1..4 + w_stop."""
                prev = s0
                for st in range(1, 5):
                    nj = 16 >> st
                    cur = stp.tile([128, nj, NB], bf16, name=f"s{st}", tag=f"s{st}")
                    filler()
                    for p in range((nj + 1) // 2):
                        npl = min(2, nj - 2 * p)
                        ps = sps.tile([128, 2, 512], f32, name="ps", tag="sp")
                        for ii in range(npl):
                            i = 2 * p + ii
                            nc.tensor.matmul(
                                ps[:, ii, :NB], lhsT=wd_sb[:, st, 0, :],
                                rhs=prev[:, 2 * i, :],
                                start=True, stop=False,
                            )
                            nc.tensor.matmul(
                                ps[:, ii, :NB], lhsT=wd_sb[:, st, 1, :],
                                rhs=prev[:, 2 * i + 1, :],
                                start=False, stop=True,
                            )
                        nc.scalar.activation(
                            out=cur[:, 2 * p : 2 * p + npl, :],
                            in_=ps[:, :npl, :NB], func=SILU,
                        )
                        yield
                    prev = cur

                # w_stop with data-stationary -> row-major [w, o] output;
                # all 4 chunks packed into one PSUM bank, single DVE copy
                s4f = prev[:, 0, :]  # [128, 511]
                outsb = outp.tile([128, 4, 128], f32, name="outsb")
                ab2 = absorb(s4f[:, 0:2], s4f[:, 0:2])
                filler()
                ps2 = ops.tile([128, 4, 128], f32, name="ps2", tag="op")
                for q, (w0, wq) in enumerate(QRANGES):
                    mmq = nc.tensor.matmul(
                        ps2[:wq, q, :],
                        lhsT=s4f[:, w0 : w0 + wq],
                        rhs=wst_sb,
                        start=True, stop=True,
                    )
                    if q == 0:
                        add_dep_helper(mmq.ins, ab2.ins, False,
                                       "absorber before first stop matmul")
                nc.vector.tensor_copy(out=outsb, in_=ps2)
                for q, (w0, wq) in enumerate(QRANGES):
                    nc.sync.dma_start(out=oqs[hh][q][:], in_=outsb[:wq, q, :])
                yield

            # software pipeline with fine-grained interleave: the stage/stop
            # units of head h alternate with the Z units of head h+1, so the
            # in-order ACT/PE streams always have independent work to backfill
            # dependency gaps
            s0s = {}
            for _ in z_units(0, s0s):
                pass
            for hh in range(HPC):
                gens = [stage_units(hh, s0s.pop(hh))]
                if hh + 1 < HPC:
                    gens.append(z_units(hh + 1, s0s))
                while gens:
                    for g in list(gens):
                        try:
                            next(g)
                        except StopIteration:
                            gens.remove(g)

    if not nc.is_finalized():
        nc.finalize()
    return nc


def _prep_host_inputs(k, pe, w_down, w_stop):
    import ml_dtypes

    bf16 = ml_dtypes.bfloat16
    k = np.asarray(k, dtype=np.float32)
    pe = np.asarray(pe, dtype=np.float32)
    w_down = np.asarray(w_down, dtype=np.float32)
    w_stop = np.asarray(w_stop, dtype=np.float32)

    # pre-permute rows so row 512*l + w = k[16w + l, :], cast to bf16 (RNE);
    # a single device-side xbar DMA-transpose then yields the l-planar layout
    k4p = np.ascontiguousarray(
        k.reshape(BH, 512, 16, D).transpose(0, 2, 1, 3).reshape(BH, S, D)
    ).astype(bf16)
    # wdt[i, half, d_in, o] = w_down[i][o, 128*half + d_in]
    wdt = np.ascontiguousarray(
        w_down.transpose(0, 2, 1).reshape(5, 2, 128, 128)
    ).astype(bf16)
    # pe0[o, j] = sum_i w_down[0][o, i] * concat(pe[2j], pe[2j+1])[i]
    pe_pairs = pe.reshape(16, 256).astype(np.float64)
    pe0 = (w_down[0].astype(np.float64) @ pe_pairs.T).astype(np.float32)
    wst = np.ascontiguousarray(w_stop.T).astype(bf16)
    return k4p, wdt, pe0, wst


def run(k, pe, w_down, w_stop, trace=False, trace_kwargs=None):
    from concourse.bass_utils import run_bass_kernel_spmd

    k4p, wdt, pe0, wst = _prep_host_inputs(k, pe, w_down, w_stop)

    if "nc" not in _BASS_CACHE:
        _BASS_CACHE["nc"] = _build_bass()
    nc = _BASS_CACHE["nc"]

    in_maps = [
        {
            "k4p": np.ascontiguousarray(k4p[HPC * c : HPC * (c + 1)]),
            "wdt": wdt,
            "pe0": pe0,
            "wst": wst,
        }
        for c in range(NCORES)
    ]
    res = run_bass_kernel_spmd(
        nc, in_maps, core_ids=list(range(NCORES)), trace=trace,
        **(trace_kwargs or {}),
    )
    out = np.empty((BH, NW, D), dtype=np.float32)
    for c in range(NCORES):
        r = res.results[c]
        for hh in range(HPC):
            row = HPC * c + hh
            out[row, 0, :] = 0.0
            at = 1
            for q, (_, wq) in enumerate(QRANGES):
                out[row, at : at + wq, :] = r[f"o{hh}_{q}"]
                at += wq
    out = out.reshape(B, H, NW, D)
    return out, res


def kernel(k, pe, w_down, w_stop):
    out, _ = run(k, pe, w_down, w_stop, trace=False)
    return out
